# revision 1
# baseline (speedup 1.0000x reference)
"""Trainium2 Bass kernel for segment max/mean pooling + Linear + ReLU.

Computes, for sorted segment ids over M lane rows:
    mx  = segment_max(lane, seg)          [N, D]
    mean= segment_sum(lane, seg)/cnt      [N, D]
    out = relu(concat([mx, mean]) @ W.T + b)   [N, OUT]

Strategy (8 NeuronCores, SPMD single program, per-core sliced inputs):
  - Rows are split across cores at group boundaries -> no collectives.
  - Per core the row stream is processed in 512-row chunks:
      * natural [128 rows, D] tiles DMA'd from HBM
      * ACT adds per-row group offsets (C*slot) so an UNMASKED running-max
        scan along the transposed stream is segmented "for free"
      * PE transposes tiles into PSUM; DVE tensor_tensor_scan computes the
        running max (carry-chained across chunks with per-chunk rebase)
      * segment sums via PE one-hot matmul into a chunk-local PSUM window
        (one-hot built on DVE from iota vs per-row slot index)
  - Segment results are extracted from scan/partial ring buffers with
    gpsimd ap_gather at host-computed end positions (128 groups at a time),
    already in the transposed [D, G] layout the output matmul needs.
  - Output: psum = maxT.T @ W1T; mean part = (sumT.T @ W2T) * invcnt;
    minus gc*rowsum(W1) correction; relu; staged in SBUF; one DMA per core.
    Host trims per-core padding groups.

The program is identical on all 8 cores; all data-dependent structure lives
in per-core input arrays (lane slice, per-row slot/offset aux, gather
indices, per-group scalars), padded to uniform shapes.
"""

from contextlib import ExitStack

import numpy as np

import concourse.bass as bass
import concourse.bacc as bacc
import concourse.tile as tile
from concourse import library_config, mybir
from concourse.bass_utils import run_bass_kernel_spmd

F32 = mybir.dt.float32
F32R = mybir.dt.float32r
BF16 = mybir.dt.bfloat16
I16 = mybir.dt.int16

N_CORES = 8
D = 128
OUT = 128
CH = 512          # rows per scan chunk
TPC = CH // 128   # tiles per chunk
NSLOT = 32        # scan ring slots (chunks) resident in SBUF
PSLOT = 32        # partial-sum ring slots
C_OFF = 16.0      # additive group separation offset
NEG = -1.0e30


# ----------------------------------------------------------------------------
# Host-side planning
# ----------------------------------------------------------------------------

def make_plan(seg, n_cores=N_CORES):
    """seg: sorted int group ids [M]. Returns uniform constants + per-core plans."""
    seg = np.asarray(seg).astype(np.int64)
    M = seg.shape[0]
    n_groups = int(seg[-1]) + 1
    cnt = np.bincount(seg, minlength=n_groups)
    assert cnt.min() >= 1, "empty group"
    assert cnt.max() <= CH, f"group of size {cnt.max()} spans >2 chunks"
    gstarts = np.zeros(n_groups + 1, dtype=np.int64)
    np.cumsum(cnt, out=gstarts[1:])

    # core boundaries at group starts
    bounds, gb = [0], [0]
    for c in range(1, n_cores):
        g = int(np.searchsorted(gstarts, M * c // n_cores))
        gb.append(g)
        bounds.append(int(gstarts[g]))
    bounds.append(M)
    gb.append(n_groups)

    rows_max = max(bounds[c + 1] - bounds[c] for c in range(n_cores))
    ROWS = ((rows_max + CH - 1) // CH) * CH
    NCH = ROWS // CH

    cores = []
    for c in range(n_cores):
        r0, r1 = bounds[c], bounds[c + 1]
        g0, g1 = gb[c], gb[c + 1]
        R, G = r1 - r0, g1 - g0
        gl = np.empty(ROWS, dtype=np.int64)
        gl[:R] = seg[r0:r1] - g0
        if R < ROWS:  # one fresh pad group per pad-containing chunk
            kfp = R // CH
            for k in range(kfp, NCH):
                a, b_ = max(R, k * CH), (k + 1) * CH
                gl[a:b_] = G + (k - kfp)
        E = int(gl[-1]) + 1
        cores.append(dict(r0=r0, r1=r1, g0=g0, g1=g1, R=R, G=G, E=E, gl=gl))

    E_MAX = max(c["E"] for c in cores)
    NFT = (E_MAX + 127) // 128
    E_PAD = NFT * 128

    W_c = 0
    for c in cores:
        gl = c["gl"]
        fgk = gl[np.arange(NCH) * CH]                 # first group per chunk
        slotrel = gl - np.repeat(fgk, CH)
        W_c = max(W_c, int(slotrel.max()) + 1)
        reb = np.zeros(NCH, dtype=np.float64)
        reb[1:] = -C_OFF * (fgk[1:] - fgk[:-1]).astype(np.float64)
        c.update(fgk=fgk, slotrel=slotrel, rebase=reb)
    assert W_c <= 128, f"W_c={W_c} too wide"
    zero_col = PSLOT * W_c

    for c in cores:
        gl, fgk, E = c["gl"], c["fgk"], c["E"]
        gids = np.arange(E)
        gstart = np.searchsorted(gl, gids, side="left")
        gend = np.searchsorted(gl, gids, side="right") - 1
        ks, ke = gstart // CH, gend // CH
        assert np.all(ke - ks <= 1), "group spans >2 chunks"
        endpos = (ke % NSLOT) * CH + (gend - ke * CH)
        p1 = (ks % PSLOT) * W_c + (gids - fgk[ks])
        p2 = np.where(ke != ks, (ke % PSLOT) * W_c + (gids - fgk[ke]), zero_col)
        assert p1.min() >= 0 and p1.max() < zero_col
        gcc = C_OFF * (gids - fgk[ke]).astype(np.float64)
        cntg = np.bincount(gl, minlength=E).astype(np.float64)
        invc = 1.0 / cntg

        def padto(a, v, dt=np.float64):
            out_ = np.full(E_PAD, v, dtype=dt)
            out_[:a.shape[0]] = a
            return out_
        c.update(
            endpos=padto(endpos, 0, np.int64),
            p1=padto(p1, zero_col, np.int64),
            p2=padto(p2, zero_col, np.int64),
            gcc=padto(gcc, 0.0),
            invc=padto(invc, 1.0),
            ks=ks, ke=ke)

    # uniform ft emission schedule: emit ft j after chunk K[j]
    K = np.zeros(NFT, dtype=np.int64)
    for j in range(NFT):
        for c in cores:
            h2 = min(j * 128 + 127, c["E"] - 1)
            K[j] = max(K[j], int(c["ke"][h2]))
    for j in range(NFT):
        for c in cores:
            kf = int(c["ks"][min(j * 128, c["E"] - 1)])
            assert K[j] < kf + NSLOT, f"scan ring too small for ft {j}"
            assert K[j] < kf + PSLOT, f"partial ring too small for ft {j}"

    return dict(M=M, n_groups=n_groups, ROWS=ROWS, NCH=NCH, W_c=W_c,
                E_MAX=E_MAX, NFT=NFT, E_PAD=E_PAD, K=K, cores=cores)


def _wrap_idx(pos, n):
    """ap_gather index layout: idx j -> [16*core + (j%16), j//16], all 8 cores."""
    assert pos.shape[0] == n and n % 16 == 0
    blk = pos.reshape(n // 16, 16).T.astype(np.int16)   # [16, n//16]
    return np.tile(blk, (8, 1))                          # [128, n//16]


def make_inputs(plan, lane, W, b):
    lane = np.ascontiguousarray(lane, dtype=np.float32)
    ROWS, NCH, W_c = plan["ROWS"], plan["NCH"], plan["W_c"]
    NFT = plan["NFT"]
    WT = np.ascontiguousarray(np.asarray(W, dtype=np.float32).T)    # [2D, OUT]
    rw1 = np.ascontiguousarray(
        np.asarray(W, dtype=np.float32)[:, :D].sum(axis=1)[None, :])  # [1, OUT]
    ident = np.eye(128, dtype=np.float32)
    iota = np.arange(W_c, dtype=np.float32)[None, :]
    assert np.abs(np.asarray(b)).max() == 0.0, "nonzero bias not implemented"
    assert np.abs(lane).max() < C_OFF / 2 - 1.0, "offset separation too small"

    in_maps = []
    for c in plan["cores"]:
        R, r0, r1 = c["R"], c["r0"], c["r1"]
        lanes = np.zeros((ROWS, D), dtype=np.float32)
        lanes[:R] = lane[r0:r1]
        aux = np.empty((ROWS, 2), dtype=np.float32)
        aux[:, 0] = c["slotrel"]
        aux[:, 1] = C_OFF * c["slotrel"]
        endidx = np.zeros((NFT, 128, 8), dtype=np.int16)
        partidx = np.zeros((NFT, 128, 16), dtype=np.int16)
        for j in range(NFT):
            endidx[j] = _wrap_idx(c["endpos"][j * 128:(j + 1) * 128], 128)
            pp = np.concatenate([c["p1"][j * 128:(j + 1) * 128],
                                 c["p2"][j * 128:(j + 1) * 128]])
            partidx[j] = _wrap_idx(pp, 256)
        in_maps.append(dict(
            lanes=lanes,
            auxrow=aux,
            rebase=np.ascontiguousarray(c["rebase"][None, :], dtype=np.float32),
            endidx=endidx,
            partidx=partidx,
            invcnt=np.ascontiguousarray(c["invc"].reshape(NFT, 128),
                                        dtype=np.float32),
            gcc=np.ascontiguousarray(-c["gcc"].reshape(NFT, 128),
                                     dtype=np.float32),
            wt=WT, rw=rw1, ident=ident, iota=np.ascontiguousarray(iota),
        ))
    return in_maps


# ----------------------------------------------------------------------------
# Device program (uniform across cores)
# ----------------------------------------------------------------------------

def build_nc(plan):
    ROWS, NCH, W_c = plan["ROWS"], plan["NCH"], plan["W_c"]
    NFT, E_PAD, K = plan["NFT"], plan["E_PAD"], plan["K"]
    PW = PSLOT * W_c + 4      # partials buffer width (incl. zero cols)

    nc = bacc.Bacc("TRN2", target_bir_lowering=False, debug=False,
                   num_devices=N_CORES)
    lanes = nc.dram_tensor("lanes", [ROWS, D], F32, kind="ExternalInput")
    auxrow = nc.dram_tensor("auxrow", [ROWS, 2], F32, kind="ExternalInput")
    rebase = nc.dram_tensor("rebase", [1, NCH], F32, kind="ExternalInput")
    endidx = nc.dram_tensor("endidx", [NFT, 128, 8], I16, kind="ExternalInput")
    partidx = nc.dram_tensor("partidx", [NFT, 128, 16], I16, kind="ExternalInput")
    invcnt = nc.dram_tensor("invcnt", [NFT, 128], F32, kind="ExternalInput")
    gcc = nc.dram_tensor("gcc", [NFT, 128], F32, kind="ExternalInput")
    wt = nc.dram_tensor("wt", [2 * D, OUT], F32, kind="ExternalInput")
    rw = nc.dram_tensor("rw", [1, OUT], F32, kind="ExternalInput")
    ident = nc.dram_tensor("ident", [128, 128], F32, kind="ExternalInput")
    iota = nc.dram_tensor("iota", [1, W_c], F32, kind="ExternalInput")
    out_c = nc.dram_tensor("out_c", [E_PAD, OUT], F32, kind="ExternalOutput")

    lanes_r = lanes[:, :].rearrange("(c t p) d -> c p t d", p=128, t=TPC)
    aux_r = auxrow[:, :].rearrange("(c t p) w -> c p t w", p=128, t=TPC)
    out_r = out_c[:, :].rearrange("(j p) o -> p j o", p=128)

    with tile.TileContext(nc) as tc, ExitStack() as ctx:
        consts = ctx.enter_context(tc.tile_pool(name="consts", bufs=1))
        bigbuf = ctx.enter_context(tc.tile_pool(name="bigbuf", bufs=1))
        xpool = ctx.enter_context(tc.tile_pool(name="xpool", bufs=8))
        xopool = ctx.enter_context(tc.tile_pool(name="xopool", bufs=10))
        auxpool = ctx.enter_context(tc.tile_pool(name="auxpool", bufs=6))
        ohpool = ctx.enter_context(tc.tile_pool(name="ohpool", bufs=12))
        carrypool = ctx.enter_context(tc.tile_pool(name="carrypool", bufs=2))
        gathpool = ctx.enter_context(tc.tile_pool(name="gathpool", bufs=2))
        finpool = ctx.enter_context(tc.tile_pool(name="finpool", bufs=2))
        psum_ch = ctx.enter_context(tc.tile_pool(name="psum_ch", bufs=5, space="PSUM"))
        psum_sm = ctx.enter_context(tc.tile_pool(name="psum_sm", bufs=2, space="PSUM"))
        psum_fin = ctx.enter_context(tc.tile_pool(name="psum_fin", bufs=1, space="PSUM"))

        # one-time constants
        ident_sb = consts.tile([128, 128], F32)
        nc.sync.dma_start(out=ident_sb[:, :], in_=ident[:, :])
        iota_sb = consts.tile([128, W_c], F32)
        nc.sync.dma_start(out=iota_sb[:, :], in_=iota[:, :].to_broadcast((128, W_c)))
        w1t_sb = consts.tile([128, OUT], F32)
        nc.sync.dma_start(out=w1t_sb[:, :], in_=wt[0:128, :])
        w2t_sb = consts.tile([128, OUT], F32)
        nc.sync.dma_start(out=w2t_sb[:, :], in_=wt[128:256, :])
        rw_sb = consts.tile([128, OUT], F32)
        nc.sync.dma_start(out=rw_sb[:, :], in_=rw[:, :].to_broadcast((128, OUT)))
        reb_sb = consts.tile([128, NCH], F32)
        nc.sync.dma_start(out=reb_sb[:, :], in_=rebase[:, :].to_broadcast((128, NCH)))
        ic_sb = consts.tile([128, NFT], F32)
        nc.sync.dma_start(out=ic_sb[:, :], in_=invcnt[:, :].rearrange("j p -> p j"))
        gcc_sb = consts.tile([128, NFT], F32)
        nc.sync.dma_start(out=gcc_sb[:, :], in_=gcc[:, :].rearrange("j p -> p j"))
        eidx_sb = consts.tile([128, NFT, 8], I16)
        nc.sync.dma_start(out=eidx_sb[:, :, :],
                          in_=endidx[:, :, :].rearrange("j p s -> p j s"))
        pidx_sb = consts.tile([128, NFT, 16], I16)
        nc.sync.dma_start(out=pidx_sb[:, :, :],
                          in_=partidx[:, :, :].rearrange("j p s -> p j s"))

        zeros_sb = consts.tile([128, CH], F32)
        nc.vector.memset(zeros_sb[:, :], 0.0)
        scan_db = bigbuf.tile([128, NSLOT * CH], F32)
        nc.vector.memset(scan_db[:, :], NEG)
        part_db = bigbuf.tile([128, PW], F32)
        nc.vector.memset(part_db[:, :], 0.0)
        staging = bigbuf.tile([128, NFT * OUT], F32)

        fts_after = {k: [] for k in range(NCH)}
        for j in range(NFT):
            fts_after[min(int(K[j]), NCH - 1)].append(j)

        def emit_ft(j):
            mx = gathpool.tile([128, 128], F32, tag="mx")
            nc.gpsimd.ap_gather(
                out_ap=mx[:, :].rearrange("p (n one) -> p n one", one=1),
                in_ap=scan_db[:, :].rearrange("p (n one) -> p n one", one=1),
                idxs_ap=eidx_sb[:, j, :],
                channels=128, num_elems=NSLOT * CH, d=1, num_idxs=128)
            pp = gathpool.tile([128, 256], F32, tag="pp")
            nc.gpsimd.ap_gather(
                out_ap=pp[:, :].rearrange("p (n one) -> p n one", one=1),
                in_ap=part_db[:, :].rearrange("p (n one) -> p n one", one=1),
                idxs_ap=pidx_sb[:, j, :],
                channels=128, num_elems=PW, d=1, num_idxs=256)
            sumT = finpool.tile([128, 128], F32, tag="sumT")
            nc.vector.tensor_add(sumT[:, :], pp[:, 0:128], pp[:, 128:256])
            fin2 = psum_fin.tile([128, 2, OUT], F32, tag="fin2")
            pmax = fin2[:, 0, :]
            pmean = fin2[:, 1, :]
            nc.tensor.matmul(pmax, mx[:, :], w1t_sb[:, :],
                             start=True, stop=True)
            nc.tensor.matmul(pmean, sumT[:, :], w2t_sb[:, :],
                             start=True, stop=True)
            m1 = finpool.tile([128, OUT], F32, tag="m1")
            nc.vector.tensor_scalar(out=m1[:, :], in0=pmean,
                                    scalar1=ic_sb[:, j:j + 1], scalar2=None,
                                    op0=mybir.AluOpType.mult)
            u = finpool.tile([128, OUT], F32, tag="u")
            nc.scalar.mul(u[:, :], rw_sb[:, :], gcc_sb[:, j:j + 1])
            t1 = finpool.tile([128, OUT], F32, tag="t1")
            nc.vector.tensor_add(t1[:, :], pmax, m1[:, :])
            t2 = finpool.tile([128, OUT], F32, tag="t2")
            nc.vector.tensor_add(t2[:, :], t1[:, :], u[:, :])
            nc.vector.tensor_scalar(out=staging[:, j * OUT:(j + 1) * OUT],
                                    in0=t2[:, :], scalar1=0.0, scalar2=None,
                                    op0=mybir.AluOpType.max)

        for k in range(NCH):
            x4 = xpool.tile([128, TPC, 128], F32, tag="x4")
            nc.sync.dma_start(out=x4[:, :, :], in_=lanes_r[k])
            a4 = auxpool.tile([128, TPC, 2], F32, tag="a4")
            nc.sync.dma_start(out=a4[:, :, :], in_=aux_r[k])
            pch = psum_ch.tile([128, CH], F32, tag="pch")
            psm = psum_sm.tile([128, W_c], F32, tag="psm")
            xos, ohs = [], []
            for t in range(TPC):
                xo = xopool.tile([128, 128], F32, tag="xo")
                nc.scalar.add(xo[:, :], x4[:, t, :], a4[:, t, 1:2])
                xos.append(xo)
            for t in range(TPC):
                oh = ohpool.tile([128, W_c], F32, tag="oh")
                nc.vector.tensor_scalar(out=oh[:, :], in0=iota_sb[:, :],
                                        scalar1=a4[:, t, 0:1], scalar2=None,
                                        op0=mybir.AluOpType.is_equal)
                ohs.append(oh)
            for t in range(TPC):
                nc.tensor.transpose(pch[:, t * 128:(t + 1) * 128], xos[t][:, :],
                                    ident_sb[:, :])
                nc.tensor.matmul(psm[:, :], x4[:, t, :], ohs[t][:, :],
                                 start=(t == 0), stop=(t == TPC - 1))
            pos = (k % NSLOT) * CH
            if k == 0:
                initial = NEG
            else:
                prev_pos = ((k - 1) % NSLOT) * CH
                cy = carrypool.tile([128, 1], F32, tag="cy")
                nc.vector.tensor_scalar(
                    out=cy[:, :],
                    in0=scan_db[:, prev_pos + CH - 1:prev_pos + CH],
                    scalar1=reb_sb[:, k:k + 1], scalar2=None,
                    op0=mybir.AluOpType.add)
                initial = cy[:, :]
            nc.vector.tensor_tensor_scan(
                out=scan_db[:, pos:pos + CH],
                data0=zeros_sb[:, :],
                data1=pch[:, :],
                initial=initial,
                op0=mybir.AluOpType.add,
                op1=mybir.AluOpType.max)
            ppos = (k % PSLOT) * W_c
            nc.vector.tensor_copy(out=part_db[:, ppos:ppos + W_c], in_=psm[:, :])
            for j in fts_after[k]:
                emit_ft(j)

        nc.sync.dma_start(
            out=out_r, in_=staging[:, :].rearrange("p (j o) -> p j o", o=OUT))

    nc.finalize()
    return nc


# ----------------------------------------------------------------------------
# Entry point
# ----------------------------------------------------------------------------

LAST_RESULT = None


def kernel(obs_encoding, lane_encoding, same_obs_mask, W, b, _debug=None):
    global LAST_RESULT
    seg = np.asarray(same_obs_mask)[:, 0]
    plan = make_plan(seg)
    in_maps = make_inputs(plan, np.asarray(lane_encoding), np.asarray(W),
                          np.asarray(b))
    nc = build_nc(plan)
    kw = dict(_debug or {})
    res = run_bass_kernel_spmd(nc, in_maps, list(range(N_CORES)), **kw)
    LAST_RESULT = res
    n_groups = plan["n_groups"]
    out = np.zeros((n_groups, OUT), dtype=np.float32)
    for ci, core in enumerate(plan["cores"]):
        g0, g1 = core["g0"], core["g1"]
        out[g0:g1] = res.results[ci]["out_c"][:g1 - g0]
    return out



# revision 10
# speedup vs baseline: 1.1590x; 1.1590x over previous
"""Trainium2 Bass kernel for segment max/mean pooling + Linear + ReLU.

Computes, for sorted segment ids over M lane rows:
    mx  = segment_max(lane, seg)          [N, D]
    mean= segment_sum(lane, seg)/cnt      [N, D]
    out = relu(concat([mx, mean]) @ W.T + b)   [N, OUT]

Strategy (8 NeuronCores, SPMD single program, per-core sliced inputs):
  - Rows split across cores at group boundaries -> no collectives.
  - Host pads every group to a multiple of 8 rows with zeros, shifts values
    by +16 (all positive, so zero pads are neutral for BOTH max and sum),
    casts to fp16, and ships the stream PRE-TRANSPOSED [128=feat, COLS].
    Within each 2048-column chunk the columns are interleaved (col = j*256+b
    for block b, lane j) so pairwise tree levels read contiguous halves
    (DVE 2x perf mode on fp16).
  - Device per chunk: 3-level pairwise tensor_tensor max-tree and sum-tree
    -> per-8-row-block max/sum [128, 256]; then two short masked scans at
    BLOCK granularity: state = (m*state) op block_val, with m=0 at
    group-start blocks. 8x fewer scan columns than a row-level scan.
  - Per 128-group tile: gpsimd ap_gather at group end-block ring columns
    (fp32), ACT converts to fp16 (sum scaled 1/64), two fp16 PE matmuls
    with W1^T / W2^T, fused (x*64/cnt)+bias via scalar_tensor_tensor where
    bias = -16*(rowsum W1 + rowsum W2) removes the shift, relu on ACT.
  - One output DMA per core; host trims padding groups.
"""

from contextlib import ExitStack

import numpy as np

import concourse.bass as bass
import concourse.bacc as bacc
import concourse.tile as tile
from concourse import library_config, mybir
from concourse.bass_utils import run_bass_kernel_spmd

F32 = mybir.dt.float32
F16 = mybir.dt.float16
I16 = mybir.dt.int16

N_CORES = 8
D = 128
OUT = 128
BLK = 8            # rows per block (group padding granularity)
CH = 4096          # padded rows per chunk
NBC = CH // BLK    # 512 block columns per chunk
NSLOTB = 10        # scan ring slots (chunks)
SH = 16.0          # positive shift added to all lane values


# ----------------------------------------------------------------------------
# Host-side planning
# ----------------------------------------------------------------------------

def make_plan(seg, n_cores=N_CORES):
    seg = np.asarray(seg).astype(np.int64)
    M = seg.shape[0]
    n_groups = int(seg[-1]) + 1
    cnt = np.bincount(seg, minlength=n_groups)
    assert cnt.min() >= 1, "empty group"
    gstarts = np.zeros(n_groups + 1, dtype=np.int64)
    np.cumsum(cnt, out=gstarts[1:])

    psz = ((cnt + BLK - 1) // BLK) * BLK
    pcum = np.zeros(n_groups + 1, dtype=np.int64)
    np.cumsum(psz, out=pcum[1:])
    total_pad = int(pcum[-1])

    gb = [0]
    for c in range(1, n_cores):
        gb.append(int(np.searchsorted(pcum, total_pad * c // n_cores)))
    gb.append(n_groups)

    rows_max = max(int(pcum[gb[c + 1]] - pcum[gb[c]]) for c in range(n_cores))
    COLS = ((rows_max + CH - 1) // CH) * CH
    NCH = COLS // CH
    NBLK = COLS // BLK
    E_MAX = max(gb[c + 1] - gb[c] for c in range(n_cores))
    NFT = (E_MAX + 127) // 128
    E_PAD = NFT * 128
    assert int(cnt.max()) <= NSLOTB * CH // 4, "group too large for ring"

    cores = []
    for c in range(n_cores):
        g0, g1 = gb[c], gb[c + 1]
        E = g1 - g0
        pc = pcum[g0:g1 + 1] - pcum[g0]       # [E+1] local padded offsets
        P = int(pc[-1])
        endblk = pc[1:] // BLK - 1            # [E] last block of each group
        ke = endblk // NBC                    # chunk containing end block
        cores.append(dict(g0=g0, g1=g1, E=E, pc=pc, P=P,
                          endblk=endblk, ke=ke))

    # uniform ft emission schedule
    K = np.zeros(NFT, dtype=np.int64)
    for j in range(NFT):
        for c in cores:
            h = min(j * 128 + 127, c["E"] - 1)
            K[j] = max(K[j], int(c["ke"][h]))
    for j in range(NFT):
        for c in cores:
            lo = j * 128
            if lo >= c["E"]:
                continue
            assert int(K[j]) - int(c["ke"][lo]) < NSLOTB, \
                f"scan ring too small for ft {j}"

    return dict(M=M, n_groups=n_groups, cnt=cnt, gstarts=gstarts,
                COLS=COLS, NCH=NCH, NBLK=NBLK, E_MAX=E_MAX, NFT=NFT,
                E_PAD=E_PAD, K=K, cores=cores)


def _wrap_idx(pos, n):
    """ap_gather index layout: idx j -> [16*core + (j%16), j//16], all 8 cores."""
    assert pos.shape[0] == n and n % 16 == 0
    blk = pos.reshape(n // 16, 16).T.astype(np.int16)   # [16, n//16]
    return np.tile(blk, (8, 1))                          # [128, n//16]


def make_inputs(plan, lane, W, b):
    lane = np.asarray(lane, dtype=np.float32)
    W = np.asarray(W, dtype=np.float32)
    assert np.abs(np.asarray(b)).max() == 0.0, "nonzero bias not implemented"
    assert np.abs(lane).max() < SH - 2.0, "shift too small for data range"
    COLS, NCH, NBLK, NFT = plan["COLS"], plan["NCH"], plan["NBLK"], plan["NFT"]
    gstarts, cnt = plan["gstarts"], plan["cnt"]

    lane16 = (lane + SH).astype(np.float16)              # [M, D]
    w1t = np.ascontiguousarray(W[:, :D].T.astype(np.float16))   # [D, OUT]
    w2t = np.ascontiguousarray(W[:, D:].T.astype(np.float16))   # [D, OUT]
    biasr = (-SH * (W[:, :D].sum(axis=1) + W[:, D:].sum(axis=1))
             ).astype(np.float32)[None, :]               # [1, OUT]
    ring = NSLOTB * NBC

    in_maps = []
    for c in plan["cores"]:
        g0, E, pc, P = c["g0"], c["E"], c["pc"], c["P"]
        # padded row -> source row map (vectorized)
        ar = np.arange(P, dtype=np.int64)
        gi = np.searchsorted(pc, ar, side="right") - 1
        off = ar - pc[gi]
        valid = off < cnt[g0 + gi]
        src = gstarts[g0 + gi] + off
        xs = np.zeros((COLS, D), dtype=np.float16)
        xs[ar[valid]] = lane16[src[valid]]
        # interleave within chunks: col j*NBC + b  <-  row b*BLK + j
        xsT = np.ascontiguousarray(
            xs.reshape(NCH, NBC, BLK, D).transpose(0, 2, 1, 3)
              .reshape(COLS, D).T)                       # [D, COLS] f16

        mrow = np.ones((1, NBLK), dtype=np.float16)
        mrow[0, pc[:-1] // BLK] = 0.0
        mrow[0, P // BLK:] = 0.0

        endpos = np.zeros(plan["E_PAD"], dtype=np.int64)
        endpos[:E] = c["endblk"] % ring
        eidx = np.zeros((NFT, 128, 8), dtype=np.int16)
        for j in range(NFT):
            eidx[j] = _wrap_idx(endpos[j * 128:(j + 1) * 128], 128)

        invcn = np.ones(plan["E_PAD"], dtype=np.float32)
        invcn[:E] = 64.0 / cnt[g0:g0 + E]

        in_maps.append(dict(
            lanesT=xsT, mrow=mrow, eidx=eidx,
            invcn=np.ascontiguousarray(invcn.reshape(NFT, 128)),
            w1t=w1t, w2t=w2t, biasr=biasr,
            ident=np.eye(128, dtype=np.float32),
        ))
    return in_maps


# ----------------------------------------------------------------------------
# Device program (uniform across cores)
# ----------------------------------------------------------------------------

def build_nc(plan):
    COLS, NCH, NFT, K = plan["COLS"], plan["NCH"], plan["NFT"], plan["K"]
    NBLK, E_PAD = plan["NBLK"], plan["E_PAD"]
    RING = NSLOTB * NBC

    nc = bacc.Bacc("TRN2", target_bir_lowering=False, debug=False,
                   num_devices=N_CORES)
    lanesT = nc.dram_tensor("lanesT", [D, COLS], F16, kind="ExternalInput")
    mrow = nc.dram_tensor("mrow", [1, NBLK], F16, kind="ExternalInput")
    eidx = nc.dram_tensor("eidx", [NFT, 128, 8], I16, kind="ExternalInput")
    invcn = nc.dram_tensor("invcn", [NFT, 128], F32, kind="ExternalInput")
    w1t = nc.dram_tensor("w1t", [D, OUT], F16, kind="ExternalInput")
    w2t = nc.dram_tensor("w2t", [D, OUT], F16, kind="ExternalInput")
    biasr = nc.dram_tensor("biasr", [1, OUT], F32, kind="ExternalInput")
    ident = nc.dram_tensor("ident", [128, 128], F32, kind="ExternalInput")
    out_c = nc.dram_tensor("out_c", [E_PAD, OUT], F32, kind="ExternalOutput")

    out_r = out_c[:, :].rearrange("(j p) o -> p j o", p=128)

    with tile.TileContext(nc) as tc, ExitStack() as ctx:
        consts = ctx.enter_context(tc.tile_pool(name="consts", bufs=1))
        bigbuf = ctx.enter_context(tc.tile_pool(name="bigbuf", bufs=1))
        xpool = ctx.enter_context(tc.tile_pool(name="xpool", bufs=3))
        mpool = ctx.enter_context(tc.tile_pool(name="mpool", bufs=3))
        t1pool = ctx.enter_context(tc.tile_pool(name="t1pool", bufs=2))
        t2pool = ctx.enter_context(tc.tile_pool(name="t2pool", bufs=2))
        t3pool = ctx.enter_context(tc.tile_pool(name="t3pool", bufs=2))
        gathpool = ctx.enter_context(tc.tile_pool(name="gathpool", bufs=2))
        finpool = ctx.enter_context(tc.tile_pool(name="finpool", bufs=2))
        psum_fin = ctx.enter_context(
            tc.tile_pool(name="psum_fin", bufs=2, space="PSUM"))

        ident_sb = consts.tile([128, 128], F32)
        nc.sync.dma_start(out=ident_sb[:, :], in_=ident[:, :])
        w1t_sb = consts.tile([D, OUT], F16)
        nc.sync.dma_start(out=w1t_sb[:, :], in_=w1t[:, :])
        w2t_sb = consts.tile([D, OUT], F16)
        nc.sync.dma_start(out=w2t_sb[:, :], in_=w2t[:, :])
        bias_sb = consts.tile([128, OUT], F32)
        nc.sync.dma_start(out=bias_sb[:, :],
                          in_=biasr[:, :].to_broadcast((128, OUT)))
        ic_sb = consts.tile([128, NFT], F32)
        nc.sync.dma_start(out=ic_sb[:, :], in_=invcn[:, :].rearrange("j p -> p j"))
        eidx_sb = consts.tile([128, NFT, 8], I16)
        nc.sync.dma_start(out=eidx_sb[:, :, :],
                          in_=eidx[:, :, :].rearrange("j p s -> p j s"))

        ringmx = bigbuf.tile([128, RING], F32)
        nc.vector.memset(ringmx[:, :], 0.0)
        ringsm = bigbuf.tile([128, RING], F32)
        nc.vector.memset(ringsm[:, :], 0.0)
        staging = bigbuf.tile([128, NFT * OUT], F32)

        MAX = mybir.AluOpType.max
        ADD = mybir.AluOpType.add
        MULT = mybir.AluOpType.mult

        fts_after = {k: [] for k in range(NCH)}
        for j in range(NFT):
            fts_after[min(int(K[j]), NCH - 1)].append(j)

        def emit_ft(j):
            mxg = gathpool.tile([128, 128], F32, tag="mxg")
            nc.gpsimd.ap_gather(
                out_ap=mxg[:, :].rearrange("p (n one) -> p n one", one=1),
                in_ap=ringmx[:, :].rearrange("p (n one) -> p n one", one=1),
                idxs_ap=eidx_sb[:, j, :],
                channels=128, num_elems=RING, d=1, num_idxs=128)
            smg = gathpool.tile([128, 128], F32, tag="smg")
            nc.gpsimd.ap_gather(
                out_ap=smg[:, :].rearrange("p (n one) -> p n one", one=1),
                in_ap=ringsm[:, :].rearrange("p (n one) -> p n one", one=1),
                idxs_ap=eidx_sb[:, j, :],
                channels=128, num_elems=RING, d=1, num_idxs=128)
            mx16 = finpool.tile([128, 128], F16, tag="mx16")
            nc.scalar.mul(mx16[:, :], mxg[:, :], 1.0)
            sm16 = finpool.tile([128, 128], F16, tag="sm16")
            nc.scalar.mul(sm16[:, :], smg[:, :], 1.0 / 64.0)
            fin2 = psum_fin.tile([128, 2, OUT], F32, tag="fin2")
            pmax = fin2[:, 0, :]
            pmean = fin2[:, 1, :]
            nc.tensor.matmul(pmean, sm16[:, :], w2t_sb[:, :],
                             start=True, stop=True)
            nc.tensor.matmul(pmax, mx16[:, :], w1t_sb[:, :],
                             start=True, stop=False)
            u = finpool.tile([128, OUT], F32, tag="u")
            nc.vector.scalar_tensor_tensor(
                out=u[:, :], in0=pmean, scalar=ic_sb[:, j:j + 1],
                in1=bias_sb[:, :], op0=MULT, op1=ADD)
            nc.tensor.matmul(pmax, ident_sb[:, :], u[:, :],
                             start=False, stop=True)
            nc.scalar.activation(staging[:, j * OUT:(j + 1) * OUT], pmax,
                                 mybir.ActivationFunctionType.Relu)

        H1, H2, H3 = CH // 2, CH // 4, CH // 8
        for k in range(NCH):
            x = xpool.tile([128, CH], F16, tag="x")
            nc.sync.dma_start(out=x[:, :], in_=lanesT[:, k * CH:(k + 1) * CH])
            m = mpool.tile([128, NBC], F16, tag="m")
            nc.sync.dma_start(
                out=m[:, :],
                in_=mrow[0:1, k * NBC:(k + 1) * NBC].to_broadcast((128, NBC)))

            a1 = t1pool.tile([128, H1], F16, tag="a1")
            nc.vector.tensor_tensor(a1[:, :], x[:, 0:H1], x[:, H1:CH], MAX)
            s1 = t1pool.tile([128, H1], F16, tag="s1")
            nc.vector.tensor_tensor(s1[:, :], x[:, 0:H1], x[:, H1:CH], ADD)
            a2 = t2pool.tile([128, H2], F16, tag="a2")
            nc.vector.tensor_tensor(a2[:, :], a1[:, 0:H2], a1[:, H2:H1], MAX)
            s2 = t2pool.tile([128, H2], F16, tag="s2")
            nc.vector.tensor_tensor(s2[:, :], s1[:, 0:H2], s1[:, H2:H1], ADD)
            a3 = t3pool.tile([128, H3], F16, tag="a3")
            nc.vector.tensor_tensor(a3[:, :], a2[:, 0:H3], a2[:, H3:H2], MAX)
            s3 = t3pool.tile([128, H3], F16, tag="s3")
            nc.vector.tensor_tensor(s3[:, :], s2[:, 0:H3], s2[:, H3:H2], ADD)

            pos = (k % NSLOTB) * NBC
            if k == 0:
                init_mx, init_sm = 0.0, 0.0
            else:
                ppos = ((k - 1) % NSLOTB) * NBC
                init_mx = ringmx[:, ppos + NBC - 1:ppos + NBC]
                init_sm = ringsm[:, ppos + NBC - 1:ppos + NBC]
            nc.vector.tensor_tensor_scan(
                out=ringmx[:, pos:pos + NBC], data0=m[:, :], data1=a3[:, :],
                initial=init_mx, op0=MULT, op1=MAX)
            nc.vector.tensor_tensor_scan(
                out=ringsm[:, pos:pos + NBC], data0=m[:, :], data1=s3[:, :],
                initial=init_sm, op0=MULT, op1=ADD)
            for j in fts_after[k]:
                emit_ft(j)

        nc.sync.dma_start(
            out=out_r, in_=staging[:, :].rearrange("p (j o) -> p j o", o=OUT))

    nc.finalize()
    return nc


# ----------------------------------------------------------------------------
# Entry point
# ----------------------------------------------------------------------------

LAST_RESULT = None


def kernel(obs_encoding, lane_encoding, same_obs_mask, W, b, _debug=None):
    global LAST_RESULT
    seg = np.asarray(same_obs_mask)[:, 0]
    plan = make_plan(seg)
    in_maps = make_inputs(plan, np.asarray(lane_encoding), np.asarray(W),
                          np.asarray(b))
    nc = build_nc(plan)
    kw = dict(_debug or {})
    res = run_bass_kernel_spmd(nc, in_maps, list(range(N_CORES)), **kw)
    LAST_RESULT = res
    n_groups = plan["n_groups"]
    out = np.zeros((n_groups, OUT), dtype=np.float32)
    for ci, core in enumerate(plan["cores"]):
        g0, g1 = core["g0"], core["g1"]
        out[g0:g1] = res.results[ci]["out_c"][:g1 - g0]
    return out


# revision 13
# speedup vs baseline: 1.3776x; 1.1886x over previous
"""Trainium2 Bass kernel for segment max/mean pooling + Linear + ReLU.

Computes, for sorted segment ids over M lane rows:
    mx  = segment_max(lane, seg)          [N, D]
    mean= segment_sum(lane, seg)/cnt      [N, D]
    out = relu(concat([mx, mean]) @ W.T + b)   [N, OUT]

Strategy (8 NeuronCores, SPMD single program, per-core sliced inputs):
  - Rows split across cores at group boundaries -> no collectives.
  - Host pads every group to a multiple of 8 rows with zeros, shifts values
    by +16 (all positive, so zero pads are neutral for BOTH max and sum),
    casts to fp16, and ships the stream PRE-TRANSPOSED [128=feat, COLS].
    Within each 2048-column chunk the columns are interleaved (col = j*256+b
    for block b, lane j) so pairwise tree levels read contiguous halves
    (DVE 2x perf mode on fp16).
  - Device per chunk: 3-level pairwise tensor_tensor max-tree and sum-tree
    -> per-8-row-block max/sum [128, 256]; then two short masked scans at
    BLOCK granularity: state = (m*state) op block_val, with m=0 at
    group-start blocks. 8x fewer scan columns than a row-level scan.
  - Per 128-group tile: gpsimd ap_gather at group end-block ring columns
    (fp32), ACT converts to fp16 (sum scaled 1/64), two fp16 PE matmuls
    with W1^T / W2^T, fused (x*64/cnt)+bias via scalar_tensor_tensor where
    bias = -16*(rowsum W1 + rowsum W2) removes the shift, relu on ACT.
  - One output DMA per core; host trims padding groups.
"""

from contextlib import ExitStack

import numpy as np

import concourse.bass as bass
import concourse.bacc as bacc
import concourse.tile as tile
from concourse import library_config, mybir
from concourse.bass_utils import run_bass_kernel_spmd

F32 = mybir.dt.float32
F16 = mybir.dt.float16
I16 = mybir.dt.int16

N_CORES = 8
D = 128
OUT = 128
BLK = 8            # rows per block (group padding granularity)
CH = 4096          # padded rows per chunk
NBC = CH // BLK    # 512 block columns per chunk
NSLOTB = 10        # scan ring slots (chunks)
SH = 16.0          # positive shift added to all lane values


# ----------------------------------------------------------------------------
# Host-side planning
# ----------------------------------------------------------------------------

def make_plan(seg, n_cores=N_CORES):
    seg = np.asarray(seg).astype(np.int64)
    M = seg.shape[0]
    n_groups = int(seg[-1]) + 1
    cnt = np.bincount(seg, minlength=n_groups)
    assert cnt.min() >= 1, "empty group"
    gstarts = np.zeros(n_groups + 1, dtype=np.int64)
    np.cumsum(cnt, out=gstarts[1:])

    psz = ((cnt + BLK - 1) // BLK) * BLK
    pcum = np.zeros(n_groups + 1, dtype=np.int64)
    np.cumsum(psz, out=pcum[1:])
    total_pad = int(pcum[-1])

    gb = [0]
    for c in range(1, n_cores):
        gb.append(int(np.searchsorted(pcum, total_pad * c // n_cores)))
    gb.append(n_groups)

    rows_max = max(int(pcum[gb[c + 1]] - pcum[gb[c]]) for c in range(n_cores))
    COLS = ((rows_max + CH - 1) // CH) * CH
    NCH = COLS // CH
    NBLK = COLS // BLK
    E_MAX = max(gb[c + 1] - gb[c] for c in range(n_cores))
    NFT = (E_MAX + 127) // 128
    E_PAD = NFT * 128
    assert int(cnt.max()) <= NSLOTB * CH // 4, "group too large for ring"

    cores = []
    for c in range(n_cores):
        g0, g1 = gb[c], gb[c + 1]
        E = g1 - g0
        pc = pcum[g0:g1 + 1] - pcum[g0]       # [E+1] local padded offsets
        P = int(pc[-1])
        endblk = pc[1:] // BLK - 1            # [E] last block of each group
        ke = endblk // NBC                    # chunk containing end block
        cores.append(dict(g0=g0, g1=g1, E=E, pc=pc, P=P,
                          endblk=endblk, ke=ke))

    # uniform ft emission schedule
    K = np.zeros(NFT, dtype=np.int64)
    for j in range(NFT):
        for c in cores:
            h = min(j * 128 + 127, c["E"] - 1)
            K[j] = max(K[j], int(c["ke"][h]))
    for j in range(NFT):
        for c in cores:
            lo = j * 128
            if lo >= c["E"]:
                continue
            assert int(K[j]) - int(c["ke"][lo]) < NSLOTB, \
                f"scan ring too small for ft {j}"

    return dict(M=M, n_groups=n_groups, cnt=cnt, gstarts=gstarts,
                COLS=COLS, NCH=NCH, NBLK=NBLK, E_MAX=E_MAX, NFT=NFT,
                E_PAD=E_PAD, K=K, cores=cores)


def _wrap_idx(pos, n):
    """ap_gather index layout: idx j -> [16*core + (j%16), j//16], all 8 cores."""
    assert pos.shape[0] == n and n % 16 == 0
    blk = pos.reshape(n // 16, 16).T.astype(np.int16)   # [16, n//16]
    return np.tile(blk, (8, 1))                          # [128, n//16]


def make_inputs(plan, lane, W, b):
    lane = np.asarray(lane, dtype=np.float32)
    W = np.asarray(W, dtype=np.float32)
    assert np.abs(np.asarray(b)).max() == 0.0, "nonzero bias not implemented"
    assert np.abs(lane).max() < SH - 2.0, "shift too small for data range"
    COLS, NCH, NBLK, NFT = plan["COLS"], plan["NCH"], plan["NBLK"], plan["NFT"]
    gstarts, cnt = plan["gstarts"], plan["cnt"]

    lane16 = (lane + SH).astype(np.float16)              # [M, D]
    w1t = np.ascontiguousarray(W[:, :D].T.astype(np.float16))   # [D, OUT]
    w2t = np.ascontiguousarray(W[:, D:].T.astype(np.float16))   # [D, OUT]
    biasr = (-SH * (W[:, :D].sum(axis=1) + W[:, D:].sum(axis=1))
             ).astype(np.float32)[None, :]               # [1, OUT]
    ring = NSLOTB * NBC

    in_maps = []
    for c in plan["cores"]:
        g0, E, pc, P = c["g0"], c["E"], c["pc"], c["P"]
        # padded row -> source row map (vectorized)
        ar = np.arange(P, dtype=np.int64)
        gi = np.searchsorted(pc, ar, side="right") - 1
        off = ar - pc[gi]
        valid = off < cnt[g0 + gi]
        src = gstarts[g0 + gi] + off
        xs = np.zeros((COLS, D), dtype=np.float16)
        xs[ar[valid]] = lane16[src[valid]]
        # interleave within chunks: col j*NBC + b  <-  row b*BLK + j
        xsT = np.ascontiguousarray(
            xs.reshape(NCH, NBC, BLK, D).transpose(0, 2, 1, 3)
              .reshape(COLS, D).T)                       # [D, COLS] f16

        mrow = np.ones((1, NBLK), dtype=np.float16)
        mrow[0, pc[:-1] // BLK] = 0.0
        mrow[0, P // BLK:] = 0.0

        endpos = np.zeros(plan["E_PAD"], dtype=np.int64)
        endpos[:E] = c["endblk"] % ring
        eidx = np.zeros((NFT, 128, 8), dtype=np.int16)
        for j in range(NFT):
            eidx[j] = _wrap_idx(endpos[j * 128:(j + 1) * 128], 128)

        invcn = np.ones(plan["E_PAD"], dtype=np.float32)
        invcn[:E] = 64.0 / cnt[g0:g0 + E]

        in_maps.append(dict(
            lanesT=xsT, mrow=mrow, eidx=eidx,
            invcn=np.ascontiguousarray(invcn.reshape(NFT, 128)),
            w1t=w1t, w2t=w2t, biasr=biasr,
            ident=np.eye(128, dtype=np.float32),
        ))
    return in_maps


# ----------------------------------------------------------------------------
# Device program (uniform across cores)
# ----------------------------------------------------------------------------

def build_nc(plan):
    COLS, NCH, NFT, K = plan["COLS"], plan["NCH"], plan["NFT"], plan["K"]
    NBLK, E_PAD = plan["NBLK"], plan["E_PAD"]
    RING = NSLOTB * NBC

    nc = bacc.Bacc("TRN2", target_bir_lowering=False, debug=False,
                   num_devices=N_CORES)
    lanesT = nc.dram_tensor("lanesT", [D, COLS], F16, kind="ExternalInput")
    mrow = nc.dram_tensor("mrow", [1, NBLK], F16, kind="ExternalInput")
    eidx = nc.dram_tensor("eidx", [NFT, 128, 8], I16, kind="ExternalInput")
    invcn = nc.dram_tensor("invcn", [NFT, 128], F32, kind="ExternalInput")
    w1t = nc.dram_tensor("w1t", [D, OUT], F16, kind="ExternalInput")
    w2t = nc.dram_tensor("w2t", [D, OUT], F16, kind="ExternalInput")
    biasr = nc.dram_tensor("biasr", [1, OUT], F32, kind="ExternalInput")
    ident = nc.dram_tensor("ident", [128, 128], F32, kind="ExternalInput")
    out_c = nc.dram_tensor("out_c", [E_PAD, OUT], F32, kind="ExternalOutput")

    out_r = out_c[:, :].rearrange("(j p) o -> p j o", p=128)

    with tile.TileContext(nc) as tc, ExitStack() as ctx:
        consts = ctx.enter_context(tc.tile_pool(name="consts", bufs=1))
        bigbuf = ctx.enter_context(tc.tile_pool(name="bigbuf", bufs=1))
        xpool = ctx.enter_context(tc.tile_pool(name="xpool", bufs=3))
        mpool = ctx.enter_context(tc.tile_pool(name="mpool", bufs=3))
        t1pool = ctx.enter_context(tc.tile_pool(name="t1pool", bufs=2))
        t2pool = ctx.enter_context(tc.tile_pool(name="t2pool", bufs=2))
        t3pool = ctx.enter_context(tc.tile_pool(name="t3pool", bufs=2))
        gathpool = ctx.enter_context(tc.tile_pool(name="gathpool", bufs=2))
        finpool = ctx.enter_context(tc.tile_pool(name="finpool", bufs=2))
        psum_fin = ctx.enter_context(
            tc.tile_pool(name="psum_fin", bufs=2, space="PSUM"))

        ident_sb = consts.tile([128, 128], F32)
        nc.sync.dma_start(out=ident_sb[:, :], in_=ident[:, :])
        ones1_sb = consts.tile([1, 128], F32)
        nc.vector.memset(ones1_sb[:, :], 1.0)
        biasr_sb = consts.tile([1, OUT], F32)
        nc.sync.dma_start(out=biasr_sb[:, :], in_=biasr[:, :])
        w1t_sb = consts.tile([D, OUT], F16)
        nc.sync.dma_start(out=w1t_sb[:, :], in_=w1t[:, :])
        w2t_sb = consts.tile([D, OUT], F16)
        nc.sync.dma_start(out=w2t_sb[:, :], in_=w2t[:, :])
        ic_sb = consts.tile([128, NFT], F32)
        nc.sync.dma_start(out=ic_sb[:, :], in_=invcn[:, :].rearrange("j p -> p j"))
        eidx_sb = consts.tile([128, NFT, 8], I16)
        nc.sync.dma_start(out=eidx_sb[:, :, :],
                          in_=eidx[:, :, :].rearrange("j p s -> p j s"))

        ringmx = bigbuf.tile([128, RING], F32)
        nc.vector.memset(ringmx[:, :], 0.0)
        ringsm = bigbuf.tile([128, RING], F32)
        nc.vector.memset(ringsm[:, :], 0.0)
        staging = bigbuf.tile([128, NFT * OUT], F32)

        MAX = mybir.AluOpType.max
        ADD = mybir.AluOpType.add
        MULT = mybir.AluOpType.mult

        fts_after = {k: [] for k in range(NCH)}
        for j in range(NFT):
            fts_after[min(int(K[j]), NCH - 1)].append(j)

        def emit_ft(j):
            smg = gathpool.tile([128, 128], F32, tag="smg")
            nc.gpsimd.ap_gather(
                out_ap=smg[:, :].rearrange("p (n one) -> p n one", one=1),
                in_ap=ringsm[:, :].rearrange("p (n one) -> p n one", one=1),
                idxs_ap=eidx_sb[:, j, :],
                channels=128, num_elems=RING, d=1, num_idxs=128)
            mxg = gathpool.tile([128, 128], F32, tag="mxg")
            nc.gpsimd.ap_gather(
                out_ap=mxg[:, :].rearrange("p (n one) -> p n one", one=1),
                in_ap=ringmx[:, :].rearrange("p (n one) -> p n one", one=1),
                idxs_ap=eidx_sb[:, j, :],
                channels=128, num_elems=RING, d=1, num_idxs=128)
            sm16 = finpool.tile([128, 128], F16, tag="sm16")
            nc.scalar.mul(sm16[:, :], smg[:, :], 1.0 / 64.0)
            mx16 = finpool.tile([128, 128], F16, tag="mx16")
            nc.scalar.mul(mx16[:, :], mxg[:, :], 1.0)
            fin2 = psum_fin.tile([128, 2, OUT], F32, tag="fin2")
            pmax = fin2[:, 0, :]
            pmean = fin2[:, 1, :]
            nc.tensor.matmul(pmean, sm16[:, :], w2t_sb[:, :],
                             start=True, stop=True)
            nc.tensor.matmul(pmax, mx16[:, :], w1t_sb[:, :],
                             start=True, stop=False)
            u = finpool.tile([128, OUT], F32, tag="u")
            nc.scalar.mul(u[:, :], pmean, ic_sb[:, j:j + 1])
            nc.tensor.matmul(pmax, ident_sb[:, :], u[:, :],
                             start=False, stop=False)
            nc.tensor.matmul(pmax, ones1_sb[:, :], biasr_sb[:, :],
                             start=False, stop=True)
            nc.scalar.activation(staging[:, j * OUT:(j + 1) * OUT], pmax,
                                 mybir.ActivationFunctionType.Relu)

        H1, H2, H3 = CH // 2, CH // 4, CH // 8
        for k in range(NCH):
            x = xpool.tile([128, CH], F16, tag="x")
            nc.sync.dma_start(out=x[:, :], in_=lanesT[:, k * CH:(k + 1) * CH])
            m = mpool.tile([128, NBC], F16, tag="m")
            nc.sync.dma_start(
                out=m[:, :],
                in_=mrow[0:1, k * NBC:(k + 1) * NBC].to_broadcast((128, NBC)))

            a1 = t1pool.tile([128, H1], F16, tag="a1")
            nc.vector.tensor_tensor(a1[:, :], x[:, 0:H1], x[:, H1:CH], MAX)
            s1 = t1pool.tile([128, H1], F16, tag="s1")
            nc.vector.tensor_tensor(s1[:, :], x[:, 0:H1], x[:, H1:CH], ADD)
            a2 = t2pool.tile([128, H2], F16, tag="a2")
            nc.vector.tensor_tensor(a2[:, :], a1[:, 0:H2], a1[:, H2:H1], MAX)
            s2 = t2pool.tile([128, H2], F16, tag="s2")
            nc.vector.tensor_tensor(s2[:, :], s1[:, 0:H2], s1[:, H2:H1], ADD)
            a3 = t3pool.tile([128, H3], F16, tag="a3")
            nc.vector.tensor_tensor(a3[:, :], a2[:, 0:H3], a2[:, H3:H2], MAX)
            s3 = t3pool.tile([128, H3], F16, tag="s3")
            nc.vector.tensor_tensor(s3[:, :], s2[:, 0:H3], s2[:, H3:H2], ADD)

            pos = (k % NSLOTB) * NBC
            if k == 0:
                init_mx, init_sm = 0.0, 0.0
            else:
                ppos = ((k - 1) % NSLOTB) * NBC
                init_mx = ringmx[:, ppos + NBC - 1:ppos + NBC]
                init_sm = ringsm[:, ppos + NBC - 1:ppos + NBC]
            nc.vector.tensor_tensor_scan(
                out=ringmx[:, pos:pos + NBC], data0=m[:, :], data1=a3[:, :],
                initial=init_mx, op0=MULT, op1=MAX)
            nc.vector.tensor_tensor_scan(
                out=ringsm[:, pos:pos + NBC], data0=m[:, :], data1=s3[:, :],
                initial=init_sm, op0=MULT, op1=ADD)
            for j in fts_after[k]:
                emit_ft(j)

        nc.sync.dma_start(
            out=out_r, in_=staging[:, :].rearrange("p (j o) -> p j o", o=OUT))

    nc.finalize()
    return nc


# ----------------------------------------------------------------------------
# Entry point
# ----------------------------------------------------------------------------

LAST_RESULT = None


def kernel(obs_encoding, lane_encoding, same_obs_mask, W, b, _debug=None):
    global LAST_RESULT
    seg = np.asarray(same_obs_mask)[:, 0]
    plan = make_plan(seg)
    in_maps = make_inputs(plan, np.asarray(lane_encoding), np.asarray(W),
                          np.asarray(b))
    nc = build_nc(plan)
    kw = dict(_debug or {})
    res = run_bass_kernel_spmd(nc, in_maps, list(range(N_CORES)), **kw)
    LAST_RESULT = res
    n_groups = plan["n_groups"]
    out = np.zeros((n_groups, OUT), dtype=np.float32)
    for ci, core in enumerate(plan["cores"]):
        g0, g1 = core["g0"], core["g1"]
        out[g0:g1] = res.results[ci]["out_c"][:g1 - g0]
    return out


# revision 16
# speedup vs baseline: 1.3825x; 1.0036x over previous
"""Trainium2 Bass kernel for segment max/mean pooling + Linear + ReLU.

Computes, for sorted segment ids over M lane rows:
    mx  = segment_max(lane, seg)          [N, D]
    mean= segment_sum(lane, seg)/cnt      [N, D]
    out = relu(concat([mx, mean]) @ W.T + b)   [N, OUT]

Strategy (8 NeuronCores, SPMD single program, per-core sliced inputs):
  - Rows split across cores at group boundaries -> no collectives.
  - Host pads every group to a multiple of 8 rows with zeros, shifts values
    by +16 (all positive, so zero pads are neutral for BOTH max and sum),
    casts to fp16, and ships the stream PRE-TRANSPOSED [128=feat, COLS].
    Within each 2048-column chunk the columns are interleaved (col = j*256+b
    for block b, lane j) so pairwise tree levels read contiguous halves
    (DVE 2x perf mode on fp16).
  - Device per chunk: 3-level pairwise tensor_tensor max-tree and sum-tree
    -> per-8-row-block max/sum [128, 256]; then two short masked scans at
    BLOCK granularity: state = (m*state) op block_val, with m=0 at
    group-start blocks. 8x fewer scan columns than a row-level scan.
  - Per 128-group tile: gpsimd ap_gather at group end-block ring columns
    (fp32), ACT converts to fp16 (sum scaled 1/64), two fp16 PE matmuls
    with W1^T / W2^T, fused (x*64/cnt)+bias via scalar_tensor_tensor where
    bias = -16*(rowsum W1 + rowsum W2) removes the shift, relu on ACT.
  - One output DMA per core; host trims padding groups.
"""

from contextlib import ExitStack

import numpy as np

import concourse.bass as bass
import concourse.bacc as bacc
import concourse.tile as tile
from concourse import library_config, mybir
from concourse.bass_utils import run_bass_kernel_spmd

F32 = mybir.dt.float32
F16 = mybir.dt.float16
I16 = mybir.dt.int16

N_CORES = 8
D = 128
OUT = 128
BLK = 8            # rows per block (group padding granularity)
CH = 4096          # padded rows per chunk
NBC = CH // BLK    # 512 block columns per chunk
NSLOTB = 10        # scan ring slots (chunks)
SH = 16.0          # positive shift added to all lane values


# ----------------------------------------------------------------------------
# Host-side planning
# ----------------------------------------------------------------------------

def make_plan(seg, n_cores=N_CORES):
    seg = np.asarray(seg).astype(np.int64)
    M = seg.shape[0]
    n_groups = int(seg[-1]) + 1
    cnt = np.bincount(seg, minlength=n_groups)
    assert cnt.min() >= 1, "empty group"
    gstarts = np.zeros(n_groups + 1, dtype=np.int64)
    np.cumsum(cnt, out=gstarts[1:])

    psz = ((cnt + BLK - 1) // BLK) * BLK
    pcum = np.zeros(n_groups + 1, dtype=np.int64)
    np.cumsum(psz, out=pcum[1:])
    total_pad = int(pcum[-1])

    gb = [0]
    for c in range(1, n_cores):
        gb.append(int(np.searchsorted(pcum, total_pad * c // n_cores)))
    gb.append(n_groups)

    rows_max = max(int(pcum[gb[c + 1]] - pcum[gb[c]]) for c in range(n_cores))
    COLS = ((rows_max + CH - 1) // CH) * CH
    NCH = COLS // CH
    NBLK = COLS // BLK
    E_MAX = max(gb[c + 1] - gb[c] for c in range(n_cores))
    NFT = (E_MAX + 127) // 128
    E_PAD = NFT * 128
    assert int(cnt.max()) <= NSLOTB * CH // 4, "group too large for ring"

    cores = []
    for c in range(n_cores):
        g0, g1 = gb[c], gb[c + 1]
        E = g1 - g0
        pc = pcum[g0:g1 + 1] - pcum[g0]       # [E+1] local padded offsets
        P = int(pc[-1])
        endblk = pc[1:] // BLK - 1            # [E] last block of each group
        ke = endblk // NBC                    # chunk containing end block
        cores.append(dict(g0=g0, g1=g1, E=E, pc=pc, P=P,
                          endblk=endblk, ke=ke))

    # uniform ft emission schedule
    K = np.zeros(NFT, dtype=np.int64)
    for j in range(NFT):
        for c in cores:
            h = min(j * 128 + 127, c["E"] - 1)
            K[j] = max(K[j], int(c["ke"][h]))
    for j in range(NFT):
        for c in cores:
            lo = j * 128
            if lo >= c["E"]:
                continue
            assert int(K[j]) - int(c["ke"][lo]) < NSLOTB, \
                f"scan ring too small for ft {j}"

    return dict(M=M, n_groups=n_groups, cnt=cnt, gstarts=gstarts,
                COLS=COLS, NCH=NCH, NBLK=NBLK, E_MAX=E_MAX, NFT=NFT,
                E_PAD=E_PAD, K=K, cores=cores)


def _wrap_idx(pos, n):
    """ap_gather index layout: idx j -> [16*core + (j%16), j//16], all 8 cores."""
    assert pos.shape[0] == n and n % 16 == 0
    blk = pos.reshape(n // 16, 16).T.astype(np.int16)   # [16, n//16]
    return np.tile(blk, (8, 1))                          # [128, n//16]


def make_inputs(plan, lane, W, b):
    lane = np.asarray(lane, dtype=np.float32)
    W = np.asarray(W, dtype=np.float32)
    assert np.abs(np.asarray(b)).max() == 0.0, "nonzero bias not implemented"
    assert np.abs(lane).max() < SH - 2.0, "shift too small for data range"
    COLS, NCH, NBLK, NFT = plan["COLS"], plan["NCH"], plan["NBLK"], plan["NFT"]
    gstarts, cnt = plan["gstarts"], plan["cnt"]

    lane16 = (lane + SH).astype(np.float16)              # [M, D]
    w1t = np.ascontiguousarray(W[:, :D].T.astype(np.float16))   # [D, OUT]
    w2t = np.ascontiguousarray(W[:, D:].T.astype(np.float16))   # [D, OUT]
    biasr = (-SH * (W[:, :D].sum(axis=1) + W[:, D:].sum(axis=1))
             ).astype(np.float32)[None, :]               # [1, OUT]
    ring = NSLOTB * NBC

    in_maps = []
    for c in plan["cores"]:
        g0, E, pc, P = c["g0"], c["E"], c["pc"], c["P"]
        # padded row -> source row map (vectorized)
        ar = np.arange(P, dtype=np.int64)
        gi = np.searchsorted(pc, ar, side="right") - 1
        off = ar - pc[gi]
        valid = off < cnt[g0 + gi]
        src = gstarts[g0 + gi] + off
        xs = np.zeros((COLS, D), dtype=np.float16)
        xs[ar[valid]] = lane16[src[valid]]
        # interleave within chunks: col j*NBC + b  <-  row b*BLK + j
        xsT = np.ascontiguousarray(
            xs.reshape(NCH, NBC, BLK, D).transpose(0, 2, 1, 3)
              .reshape(COLS, D).T)                       # [D, COLS] f16

        mrow1 = np.ones((1, NBLK), dtype=np.float16)
        mrow1[0, pc[:-1] // BLK] = 0.0
        mrow1[0, P // BLK:] = 0.0
        mrow = np.ascontiguousarray(np.broadcast_to(mrow1, (128, NBLK)))

        endpos = np.zeros(plan["E_PAD"], dtype=np.int64)
        endpos[:E] = c["endblk"] % ring
        eidx = np.zeros((NFT, 128, 8), dtype=np.int16)
        for j in range(NFT):
            eidx[j] = _wrap_idx(endpos[j * 128:(j + 1) * 128], 128)

        invcn = np.ones(plan["E_PAD"], dtype=np.float32)
        invcn[:E] = 64.0 / cnt[g0:g0 + E]

        in_maps.append(dict(
            lanesT=xsT, mrow=mrow, eidx=eidx,
            invcn=np.ascontiguousarray(invcn.reshape(NFT, 128)),
            w1t=w1t, w2t=w2t, biasr=biasr,
            ident=np.eye(128, dtype=np.float32),
        ))
    return in_maps


# ----------------------------------------------------------------------------
# Device program (uniform across cores)
# ----------------------------------------------------------------------------

def build_nc(plan):
    COLS, NCH, NFT, K = plan["COLS"], plan["NCH"], plan["NFT"], plan["K"]
    NBLK, E_PAD = plan["NBLK"], plan["E_PAD"]
    RING = NSLOTB * NBC

    nc = bacc.Bacc("TRN2", target_bir_lowering=False, debug=False,
                   num_devices=N_CORES)
    lanesT = nc.dram_tensor("lanesT", [D, COLS], F16, kind="ExternalInput")
    mrow = nc.dram_tensor("mrow", [128, NBLK], F16, kind="ExternalInput")
    eidx = nc.dram_tensor("eidx", [NFT, 128, 8], I16, kind="ExternalInput")
    invcn = nc.dram_tensor("invcn", [NFT, 128], F32, kind="ExternalInput")
    w1t = nc.dram_tensor("w1t", [D, OUT], F16, kind="ExternalInput")
    w2t = nc.dram_tensor("w2t", [D, OUT], F16, kind="ExternalInput")
    biasr = nc.dram_tensor("biasr", [1, OUT], F32, kind="ExternalInput")
    ident = nc.dram_tensor("ident", [128, 128], F32, kind="ExternalInput")
    out_c = nc.dram_tensor("out_c", [E_PAD, OUT], F32, kind="ExternalOutput")

    out_r = out_c[:, :].rearrange("(j p) o -> p j o", p=128)

    with tile.TileContext(nc) as tc, ExitStack() as ctx:
        consts = ctx.enter_context(tc.tile_pool(name="consts", bufs=1))
        bigbuf = ctx.enter_context(tc.tile_pool(name="bigbuf", bufs=1))
        xpool = ctx.enter_context(tc.tile_pool(name="xpool", bufs=3))
        mpool = ctx.enter_context(tc.tile_pool(name="mpool", bufs=3))
        t1pool = ctx.enter_context(tc.tile_pool(name="t1pool", bufs=2))
        t2pool = ctx.enter_context(tc.tile_pool(name="t2pool", bufs=2))
        t3pool = ctx.enter_context(tc.tile_pool(name="t3pool", bufs=2))
        gathpool = ctx.enter_context(tc.tile_pool(name="gathpool", bufs=2))
        finpool = ctx.enter_context(tc.tile_pool(name="finpool", bufs=2))
        psum_fin = ctx.enter_context(
            tc.tile_pool(name="psum_fin", bufs=2, space="PSUM"))

        ident_sb = consts.tile([128, 128], F32)
        nc.sync.dma_start(out=ident_sb[:, :], in_=ident[:, :])
        ones1_sb = consts.tile([1, 128], F32)
        nc.vector.memset(ones1_sb[:, :], 1.0)
        biasr_sb = consts.tile([1, OUT], F32)
        nc.sync.dma_start(out=biasr_sb[:, :], in_=biasr[:, :])
        w1t_sb = consts.tile([D, OUT], F16)
        nc.sync.dma_start(out=w1t_sb[:, :], in_=w1t[:, :])
        w2t_sb = consts.tile([D, OUT], F16)
        nc.sync.dma_start(out=w2t_sb[:, :], in_=w2t[:, :])
        ic_sb = consts.tile([128, NFT], F32)
        nc.sync.dma_start(out=ic_sb[:, :], in_=invcn[:, :].rearrange("j p -> p j"))
        eidx_sb = consts.tile([128, NFT, 8], I16)
        nc.sync.dma_start(out=eidx_sb[:, :, :],
                          in_=eidx[:, :, :].rearrange("j p s -> p j s"))

        ringmx = bigbuf.tile([128, RING], F32)
        nc.vector.memset(ringmx[:, :], 0.0)
        ringsm = bigbuf.tile([128, RING], F32)
        nc.vector.memset(ringsm[:, :], 0.0)
        staging = bigbuf.tile([128, NFT * OUT], F32)

        MAX = mybir.AluOpType.max
        ADD = mybir.AluOpType.add
        MULT = mybir.AluOpType.mult

        fts_after = {k: [] for k in range(NCH)}
        for j in range(NFT):
            fts_after[min(int(K[j]), NCH - 1)].append(j)

        def emit_ft(j):
            smg = gathpool.tile([128, 128], F32, tag="smg")
            nc.gpsimd.ap_gather(
                out_ap=smg[:, :].rearrange("p (n one) -> p n one", one=1),
                in_ap=ringsm[:, :].rearrange("p (n one) -> p n one", one=1),
                idxs_ap=eidx_sb[:, j, :],
                channels=128, num_elems=RING, d=1, num_idxs=128)
            mxg = gathpool.tile([128, 128], F32, tag="mxg")
            nc.gpsimd.ap_gather(
                out_ap=mxg[:, :].rearrange("p (n one) -> p n one", one=1),
                in_ap=ringmx[:, :].rearrange("p (n one) -> p n one", one=1),
                idxs_ap=eidx_sb[:, j, :],
                channels=128, num_elems=RING, d=1, num_idxs=128)
            sm16 = finpool.tile([128, 128], F16, tag="sm16")
            nc.scalar.mul(sm16[:, :], smg[:, :], 1.0 / 64.0)
            mx16 = finpool.tile([128, 128], F16, tag="mx16")
            nc.scalar.mul(mx16[:, :], mxg[:, :], 1.0)
            fin2 = psum_fin.tile([128, 2, OUT], F32, tag="fin2")
            pmax = fin2[:, 0, :]
            pmean = fin2[:, 1, :]
            nc.tensor.matmul(pmean, sm16[:, :], w2t_sb[:, :],
                             start=True, stop=True)
            nc.tensor.matmul(pmax, mx16[:, :], w1t_sb[:, :],
                             start=True, stop=False)
            u = finpool.tile([128, OUT], F32, tag="u")
            nc.scalar.mul(u[:, :], pmean, ic_sb[:, j:j + 1])
            nc.tensor.matmul(pmax, ident_sb[:, :], u[:, :],
                             start=False, stop=False)
            nc.tensor.matmul(pmax, ones1_sb[:, :], biasr_sb[:, :],
                             start=False, stop=True)
            nc.scalar.activation(staging[:, j * OUT:(j + 1) * OUT], pmax,
                                 mybir.ActivationFunctionType.Relu)

        H1, H2, H3 = CH // 2, CH // 4, CH // 8
        for k in range(NCH):
            x = xpool.tile([128, CH], F16, tag="x")
            nc.sync.dma_start(out=x[:, :], in_=lanesT[:, k * CH:(k + 1) * CH])
            m = mpool.tile([128, NBC], F16, tag="m")
            nc.sync.dma_start(out=m[:, :],
                              in_=mrow[:, k * NBC:(k + 1) * NBC])

            a1 = t1pool.tile([128, H1], F16, tag="a1")
            nc.vector.tensor_tensor(a1[:, :], x[:, 0:H1], x[:, H1:CH], MAX)
            s1 = t1pool.tile([128, H1], F16, tag="s1")
            nc.vector.tensor_tensor(s1[:, :], x[:, 0:H1], x[:, H1:CH], ADD)
            a2 = t2pool.tile([128, H2], F16, tag="a2")
            nc.vector.tensor_tensor(a2[:, :], a1[:, 0:H2], a1[:, H2:H1], MAX)
            s2 = t2pool.tile([128, H2], F16, tag="s2")
            nc.vector.tensor_tensor(s2[:, :], s1[:, 0:H2], s1[:, H2:H1], ADD)
            a3 = t3pool.tile([128, H3], F16, tag="a3")
            nc.vector.tensor_tensor(a3[:, :], a2[:, 0:H3], a2[:, H3:H2], MAX)
            s3 = t3pool.tile([128, H3], F16, tag="s3")
            nc.vector.tensor_tensor(s3[:, :], s2[:, 0:H3], s2[:, H3:H2], ADD)

            pos = (k % NSLOTB) * NBC
            if k == 0:
                init_mx, init_sm = 0.0, 0.0
            else:
                ppos = ((k - 1) % NSLOTB) * NBC
                init_mx = ringmx[:, ppos + NBC - 1:ppos + NBC]
                init_sm = ringsm[:, ppos + NBC - 1:ppos + NBC]
            nc.vector.tensor_tensor_scan(
                out=ringmx[:, pos:pos + NBC], data0=m[:, :], data1=a3[:, :],
                initial=init_mx, op0=MULT, op1=MAX)
            nc.vector.tensor_tensor_scan(
                out=ringsm[:, pos:pos + NBC], data0=m[:, :], data1=s3[:, :],
                initial=init_sm, op0=MULT, op1=ADD)
            for j in fts_after[k]:
                emit_ft(j)

        nc.sync.dma_start(
            out=out_r, in_=staging[:, :].rearrange("p (j o) -> p j o", o=OUT))

    nc.finalize()
    return nc


# ----------------------------------------------------------------------------
# Entry point
# ----------------------------------------------------------------------------

LAST_RESULT = None


def kernel(obs_encoding, lane_encoding, same_obs_mask, W, b, _debug=None):
    global LAST_RESULT
    seg = np.asarray(same_obs_mask)[:, 0]
    plan = make_plan(seg)
    in_maps = make_inputs(plan, np.asarray(lane_encoding), np.asarray(W),
                          np.asarray(b))
    nc = build_nc(plan)
    kw = dict(_debug or {})
    res = run_bass_kernel_spmd(nc, in_maps, list(range(N_CORES)), **kw)
    LAST_RESULT = res
    n_groups = plan["n_groups"]
    out = np.zeros((n_groups, OUT), dtype=np.float32)
    for ci, core in enumerate(plan["cores"]):
        g0, g1 = core["g0"], core["g1"]
        out[g0:g1] = res.results[ci]["out_c"][:g1 - g0]
    return out
